# revision 11
# baseline (speedup 1.0000x reference)
"""Locally-connected Conv2d (unique weights per output location) on 8 trn2 cores.

Problem (hardcoded): x [256,1,280,280] f32, weight [12800,1,28,28] f32,
bias [12800,1] f32 -> out [256,128,10,10] f32.  kernel 28x28, stride 28
(non-overlapping patches), 10x10=100 locations, 128 filters.

Per location l the computation is a plain matmul:
    out[b, f, l] = sum_k patch[b, l, k] * w[f, l, k] + bias[f, l],  k in [0,784)

Strategy: shard the 100 locations across 8 cores (pad to 104 = 8*13).
Host-side we repack x into k-major patch layout and weights into k-major
filter layout (both fp16 to halve HBM traffic; accumulation is fp32 in
PSUM), so the device does nothing but streaming matmuls:
    per location: 7 accumulating matmuls [113k x 128f]^T @ [113k x 256b]
The 113th contraction row folds the bias in (x side = 1.0, w side = bias
on chunk 0 and 0 on the rest), so no separate bias add is needed.
"""

import numpy as np

import concourse.bass as bass
import concourse.mybir as mybir
from concourse import bass_utils
from concourse.tile import TileContext
from concourse.vector_clock import ScopedClock


def _split_drain_and_barrier(self, tick_clock, wait_clock):
    """TileContext._drain_and_barrier with the tail drain's sem waits split
    across several drain instructions: this walrus build caps the number of
    sync-wait commands a single instruction may carry."""
    drain_inst = self.nc.sync.drain()
    wait_clock.add_sem_waits(
        drain_inst.ins, ScopedClock({None: tick_clock.global_clock}))
    mi = drain_inst.ins
    if mi.sync_info is not None and mi.sync_info.on_wait:
        waits = list(mi.sync_info.on_wait)
        ups = list(mi.sync_info.on_update or [])
        mi.sync_info = mybir.SyncInfo(on_wait=waits[:1], on_update=ups)
        for w in waits[1:]:
            extra = self.nc.sync.drain()
            extra.ins.sync_info = mybir.SyncInfo(on_wait=[w], on_update=[])
    self.nc.all_engine_barrier()
    assert self.sems is not None
    popped = self.nc._tile_sem_poison_stack.pop()
    assert popped is self._sem_poison
    self.nc.clear_and_free_semaphores(list(self.sems.allocated().values()))
    self.nc.all_engine_barrier()


TileContext._drain_and_barrier = _split_drain_and_barrier

B = 256       # batch
NF = 128      # filters
HS = WS = 10  # output spatial
L = HS * WS   # locations
KH = KW = 28  # kernel == stride (non-overlapping)
K = KH * KW   # contraction length per location (784)
NCORES = 8
LPC = 13      # locations per core (8*13 = 104 >= 100, tail zero-padded)
LPAD = NCORES * LPC
KC = 7        # contraction chunks
KP = 113      # partitions per chunk: 112 real k (kh%4, kw) + 1 bias row

_CACHED = {}


def _build_bass():
    nc = bass.Bass(trn_type="TRN2")
    xk = nc.dram_tensor("xk", [LPC, KP, KC, B], mybir.dt.float16,
                        kind="ExternalInput")
    wk = nc.dram_tensor("wk", [LPC, KP, KC, NF], mybir.dt.float16,
                        kind="ExternalInput")
    # This walrus build allows only ONE sync-wait per DMA instruction, and
    # Tile adds a lane-reuse wait to every DMA past the 8th on a lane group.
    # So data-dependent stores must (a) ride the SWDGE lane group (loads use
    # the 8 HWDGE lanes) and (b) number at most 8.  Batch 2 locations per
    # store; separate DRAM tensors avoid per-tensor WAW chaining.
    NPAIR = (LPC + 1) // 2
    outs = [nc.dram_tensor(f"out{p}", [NF, min(2, LPC - 2 * p), B],
                           mybir.dt.float32, kind="ExternalOutput")
            for p in range(NPAIR)]

    # bufs=LPC: every tile gets a fresh slot for the whole run, so no
    # instruction ever needs a slot-release wait (this walrus build allows
    # only 1 sync-wait on DMA instructions and 2 on engine instructions).
    with TileContext(nc) as tc:
        with (
            tc.tile_pool(name="xp", bufs=LPC) as xpool,
            tc.tile_pool(name="wp", bufs=LPC) as wpool,
            tc.tile_pool(name="op", bufs=NPAIR) as opool,
            # 2 locations share one PSUM bank: NPAIR=7 tiles <= 8 banks, so
            # banks are never reused and matmuls need no release wait
            # (the MM struct also only allows one sync-wait).
            tc.tile_pool(name="ps", bufs=NPAIR, space="PSUM") as pspool,
        ):
            for p in range(NPAIR):
                npl = outs[p].shape[1]
                ps = pspool.tile([NF, npl, B], mybir.dt.float32)
                for half in range(npl):
                    l = 2 * p + half
                    x_t = xpool.tile([KP, KC, B], mybir.dt.float16)
                    w_t = wpool.tile([KP, KC, NF], mybir.dt.float16)
                    nc.sync.dma_start(x_t[:], xk[l])
                    nc.sync.dma_start(w_t[:], wk[l])
                    for c in range(KC):
                        nc.tensor.matmul(ps[:, half, :], w_t[:, c, :],
                                         x_t[:, c, :],
                                         start=(c == 0), stop=(c == KC - 1))
                o_t = opool.tile([NF, npl, B], mybir.dt.float32)
                nc.vector.tensor_copy(o_t[:], ps[:])
                nc.gpsimd.dma_start(outs[p][:], o_t[:])
    return nc


def _pack_inputs(x, weight, bias):
    # x: [B,1,280,280] f32.  rows = i*28 + kh, kh = c*4 + khm; cols = j*28 + kw
    # xk[l=(i,j), p, c, b] fp16 with p = khm*28 + kw for p<112, p=112 -> 1.0
    xh = x.astype(np.float16).reshape(B, HS, KC, 4, WS, KW)
    # (b, i, c, khm, j, kw) -> (i, j, khm, kw, c, b)
    xt = np.ascontiguousarray(xh.transpose(1, 4, 3, 5, 2, 0))
    xk = np.empty((LPAD, KP, KC, B), np.float16)
    xk[:L, :112] = xt.reshape(L, 112, KC, B)
    xk[L:, :112] = 0
    xk[:, 112] = 1.0

    # weight: [NF*L, 1, 28, 28] -> [f, l, c, khm, kw] -> [l, (khm,kw), c, f]
    wh = weight.astype(np.float16).reshape(NF, L, KC, 4, KW)
    wt = np.ascontiguousarray(wh.transpose(1, 3, 4, 2, 0)).reshape(L, 112, KC, NF)
    wk = np.zeros((LPAD, KP, KC, NF), np.float16)
    wk[:L, :112] = wt
    # bias row: only chunk 0 carries it (x side is 1.0 in every chunk)
    wk[:L, 112, 0, :] = bias.astype(np.float16).reshape(NF, L).T

    in_maps = []
    for c in range(NCORES):
        sl = slice(c * LPC, (c + 1) * LPC)
        in_maps.append({"xk": xk[sl], "wk": wk[sl]})
    return in_maps


def run(x, weight, bias, **run_kwargs):
    """Build+run; returns (output, BassKernelResults)."""
    if "nc" not in _CACHED:
        _CACHED["nc"] = _build_bass()
    nc = _CACHED["nc"]
    in_maps = _pack_inputs(x, weight, bias)
    res = bass_utils.run_bass_kernel_spmd(
        nc, in_maps, core_ids=list(range(NCORES)), **run_kwargs)
    # per core: out{p} is [NF, npl, B]; concat pairs -> [NF, LPC, B]
    outs = np.stack([
        np.concatenate([r[f"out{p}"] for p in range((LPC + 1) // 2)], axis=1)
        for r in res.results])                        # [8, NF, LPC, B]
    outs = outs.transpose(0, 2, 1, 3).reshape(LPAD, NF, B)[:L]  # [l, f, b]
    out = np.ascontiguousarray(outs.transpose(2, 1, 0)).reshape(B, NF, HS, WS)
    return out.astype(np.float32), res


def kernel(x, weight, bias):
    out, _ = run(x, weight, bias)
    return out


# revision 15
# speedup vs baseline: 3.0729x; 3.0729x over previous
"""Locally-connected Conv2d (unique weights per output location) on 8 trn2 cores.

Problem (hardcoded): x [256,1,280,280] f32, weight [12800,1,28,28] f32,
bias [12800,1] f32 -> out [256,128,10,10] f32.  kernel 28x28, stride 28
(non-overlapping patches), 10x10=100 locations, 128 filters.

Per location l the computation is a plain matmul:
    out[b, f, l] = sum_k patch[b, l, k] * w[f, l, k] + bias[f, l],  k in [0,784)

Strategy: shard the 100 locations across 8 cores (pad to 104 = 8*13).
Host-side we repack x into k-major patch layout and weights into k-major
filter layout (both fp16 to halve HBM traffic; accumulation is fp32 in
PSUM), so the device does nothing but streaming matmuls:
    per location: 7 accumulating matmuls [113k x 128f]^T @ [113k x 256b]
The 113th contraction row folds the bias in (x side = 1.0, w side = bias
on chunk 0 and 0 on the rest), so no separate bias add is needed.
"""

import numpy as np

import concourse.bass as bass
import concourse.mybir as mybir
from concourse import bass_utils
from concourse.tile import TileContext
from concourse.vector_clock import ScopedClock


def _split_drain_and_barrier(self, tick_clock, wait_clock):
    """TileContext._drain_and_barrier with the tail drain's sem waits split
    across several drain instructions: this walrus build caps the number of
    sync-wait commands a single instruction may carry."""
    drain_inst = self.nc.sync.drain()
    wait_clock.add_sem_waits(
        drain_inst.ins, ScopedClock({None: tick_clock.global_clock}))
    mi = drain_inst.ins
    if mi.sync_info is not None and mi.sync_info.on_wait:
        waits = list(mi.sync_info.on_wait)
        ups = list(mi.sync_info.on_update or [])
        mi.sync_info = mybir.SyncInfo(on_wait=waits[:1], on_update=ups)
        for w in waits[1:]:
            extra = self.nc.sync.drain()
            extra.ins.sync_info = mybir.SyncInfo(on_wait=[w], on_update=[])
    self.nc.all_engine_barrier()
    assert self.sems is not None
    popped = self.nc._tile_sem_poison_stack.pop()
    assert popped is self._sem_poison
    self.nc.clear_and_free_semaphores(list(self.sems.allocated().values()))
    self.nc.all_engine_barrier()


TileContext._drain_and_barrier = _split_drain_and_barrier

B = 256       # batch
NF = 128      # filters
HS = WS = 10  # output spatial
L = HS * WS   # locations
KH = KW = 28  # kernel == stride (non-overlapping)
K = KH * KW   # contraction length per location (784)
NCORES = 8
LPC = 13      # locations per core (8*13 = 104 >= 100, tail zero-padded)
LPAD = NCORES * LPC
KC = 7        # contraction chunks
KP = 113      # partitions per chunk: 112 real k (kh%4, kw) + 1 bias row

_CACHED = {}


def _build_bass():
    nc = bass.Bass(trn_type="TRN2")
    xk = nc.dram_tensor("xk", [LPC, KP, KC, B], mybir.dt.float16,
                        kind="ExternalInput")
    wk = nc.dram_tensor("wk", [LPC, KP, KC, NF], mybir.dt.float16,
                        kind="ExternalInput")
    # This walrus build allows only ONE sync-wait per DMA instruction, and
    # Tile adds a lane-reuse wait to every DMA past the 8th on a lane group.
    # So data-dependent stores must (a) ride the SWDGE lane group (loads use
    # the 8 HWDGE lanes) and (b) number at most 8.  Batch 2 locations per
    # store; separate DRAM tensors avoid per-tensor WAW chaining.
    NPAIR = (LPC + 1) // 2
    outs = [nc.dram_tensor(f"out{p}", [NF, min(2, LPC - 2 * p), B],
                           mybir.dt.float32, kind="ExternalOutput")
            for p in range(NPAIR)]

    # bufs=LPC: every tile gets a fresh slot for the whole run, so no
    # instruction ever needs a slot-release wait (this walrus build allows
    # only 1 sync-wait on DMA instructions and 2 on engine instructions).
    with TileContext(nc) as tc:
        # All DMA via SWDGE (nc.gpsimd): the HWDGE rings in this runtime feed
        # a single SDMA engine (~27 GB/s); SWDGE sprays all 16 (~400 GB/s).
        with (
            tc.tile_pool(name="xp", bufs=LPC) as xpool,
            tc.tile_pool(name="wp", bufs=LPC) as wpool,
            tc.tile_pool(name="op", bufs=NPAIR) as opool,
            tc.tile_pool(name="cr", bufs=NPAIR) as crpool,
            # 2 locations share one PSUM bank: NPAIR=7 tiles <= 8 banks, so
            # banks are never reused and matmuls need no release wait
            # (the MM struct also only allows one sync-wait).
            tc.tile_pool(name="ps", bufs=NPAIR, space="PSUM") as pspool,
        ):
            for p in range(NPAIR):
                npl = outs[p].shape[1]
                ps = pspool.tile([NF, npl, B], mybir.dt.float32)
                for half in range(npl):
                    l = 2 * p + half
                    x_t = xpool.tile([KP, KC, B], mybir.dt.float16)
                    w_t = wpool.tile([KP, KC, NF], mybir.dt.float16)
                    nc.gpsimd.dma_start(x_t[:], xk[l])
                    nc.gpsimd.dma_start(w_t[:], wk[l])
                    for c in range(KC):
                        nc.tensor.matmul(ps[:, half, :], w_t[:, c, :],
                                         x_t[:, c, :],
                                         start=(c == 0), stop=(c == KC - 1))
                o_t = opool.tile([NF, npl, B], mybir.dt.float32)
                nc.vector.tensor_copy(o_t[:], ps[:])
                # Carrier: a Pool-engine read of o_t makes the Pool engine
                # wait on the DVE copy, so the store DMA below (also Pool)
                # gets that wait elided and stays within 1 sync-wait.
                scratch = crpool.tile([1, 2], mybir.dt.float32, tag="scratch")
                nc.gpsimd.tensor_copy(scratch[:], o_t[0:1, 0, 0:2])
                nc.gpsimd.dma_start(outs[p][:], o_t[:])
    return nc


def _pack_inputs(x, weight, bias):
    # x: [B,1,280,280] f32.  rows = i*28 + kh, kh = c*4 + khm; cols = j*28 + kw
    # xk[l=(i,j), p, c, b] fp16 with p = khm*28 + kw for p<112, p=112 -> 1.0
    xh = x.astype(np.float16).reshape(B, HS, KC, 4, WS, KW)
    # (b, i, c, khm, j, kw) -> (i, j, khm, kw, c, b)
    xt = np.ascontiguousarray(xh.transpose(1, 4, 3, 5, 2, 0))
    xk = np.empty((LPAD, KP, KC, B), np.float16)
    xk[:L, :112] = xt.reshape(L, 112, KC, B)
    xk[L:, :112] = 0
    xk[:, 112] = 1.0

    # weight: [NF*L, 1, 28, 28] -> [f, l, c, khm, kw] -> [l, (khm,kw), c, f]
    wh = weight.astype(np.float16).reshape(NF, L, KC, 4, KW)
    wt = np.ascontiguousarray(wh.transpose(1, 3, 4, 2, 0)).reshape(L, 112, KC, NF)
    wk = np.zeros((LPAD, KP, KC, NF), np.float16)
    wk[:L, :112] = wt
    # bias row: only chunk 0 carries it (x side is 1.0 in every chunk)
    wk[:L, 112, 0, :] = bias.astype(np.float16).reshape(NF, L).T

    in_maps = []
    for c in range(NCORES):
        sl = slice(c * LPC, (c + 1) * LPC)
        in_maps.append({"xk": xk[sl], "wk": wk[sl]})
    return in_maps


def run(x, weight, bias, **run_kwargs):
    """Build+run; returns (output, BassKernelResults)."""
    if "nc" not in _CACHED:
        _CACHED["nc"] = _build_bass()
    nc = _CACHED["nc"]
    in_maps = _pack_inputs(x, weight, bias)
    res = bass_utils.run_bass_kernel_spmd(
        nc, in_maps, core_ids=list(range(NCORES)), **run_kwargs)
    # per core: out{p} is [NF, npl, B]; concat pairs -> [NF, LPC, B]
    outs = np.stack([
        np.concatenate([r[f"out{p}"] for p in range((LPC + 1) // 2)], axis=1)
        for r in res.results])                        # [8, NF, LPC, B]
    outs = outs.transpose(0, 2, 1, 3).reshape(LPAD, NF, B)[:L]  # [l, f, b]
    out = np.ascontiguousarray(outs.transpose(2, 1, 0)).reshape(B, NF, HS, WS)
    return out.astype(np.float32), res


def kernel(x, weight, bias):
    out, _ = run(x, weight, bias)
    return out


# revision 16
# speedup vs baseline: 3.2971x; 1.0730x over previous
"""Locally-connected Conv2d (unique weights per output location) on 8 trn2 cores.

Problem (hardcoded): x [256,1,280,280] f32, weight [12800,1,28,28] f32,
bias [12800,1] f32 -> out [256,128,10,10] f32.  kernel 28x28, stride 28
(non-overlapping patches), 10x10=100 locations, 128 filters.

Per location l the computation is a plain matmul:
    out[b, f, l] = sum_k patch[b, l, k] * w[f, l, k] + bias[f, l],  k in [0,784)

Strategy: shard the 100 locations across 8 cores (pad to 104 = 8*13).
Host-side we repack x into k-major patch layout and weights into k-major
filter layout (both fp16 to halve HBM traffic; accumulation is fp32 in
PSUM), so the device does nothing but streaming matmuls:
    per location: 7 accumulating matmuls [113k x 128f]^T @ [113k x 256b]
The 113th contraction row folds the bias in (x side = 1.0, w side = bias
on chunk 0 and 0 on the rest), so no separate bias add is needed.

Environment-driven constraints (this walrus build / axon runtime):
  - each DMA instruction may carry at most ONE sync-wait; matmul ONE;
    Pool tensor-copy ONE; DVE copy TWO.
  - Tile adds a lane-reuse wait to any DMA past the 8th on a lane group,
    so we use exactly 8 SWDGE transfers (3 x-blocks, 3 w-blocks, 2 output
    stores) and never reuse a lane.
  - HWDGE (nc.sync) feeds a single SDMA engine here (~27 GB/s); SWDGE
    (nc.gpsimd) sprays all 16 (~400 GB/s) -> all DMA goes through gpsimd.
"""

import numpy as np

import concourse.bass as bass
import concourse.mybir as mybir
from concourse import bass_utils
from concourse.tile import TileContext
from concourse.vector_clock import ScopedClock


def _split_drain_and_barrier(self, tick_clock, wait_clock):
    """TileContext._drain_and_barrier with the tail drain's sem waits split
    across several drain instructions: this walrus build caps the number of
    sync-wait commands a single instruction may carry."""
    drain_inst = self.nc.sync.drain()
    wait_clock.add_sem_waits(
        drain_inst.ins, ScopedClock({None: tick_clock.global_clock}))
    mi = drain_inst.ins
    if mi.sync_info is not None and mi.sync_info.on_wait:
        waits = list(mi.sync_info.on_wait)
        ups = list(mi.sync_info.on_update or [])
        mi.sync_info = mybir.SyncInfo(on_wait=waits[:1], on_update=ups)
        for w in waits[1:]:
            extra = self.nc.sync.drain()
            extra.ins.sync_info = mybir.SyncInfo(on_wait=[w], on_update=[])
    self.nc.all_engine_barrier()
    assert self.sems is not None
    popped = self.nc._tile_sem_poison_stack.pop()
    assert popped is self._sem_poison
    self.nc.clear_and_free_semaphores(list(self.sems.allocated().values()))
    self.nc.all_engine_barrier()


TileContext._drain_and_barrier = _split_drain_and_barrier

B = 256       # batch
NF = 128      # filters
HS = WS = 10  # output spatial
L = HS * WS   # locations
KH = KW = 28  # kernel == stride (non-overlapping)
K = KH * KW   # contraction length per location (784)
NCORES = 8
LPC = 13      # locations per core (8*13 = 104 >= 100, tail zero-padded)
LPAD = NCORES * LPC
KC = 7        # contraction chunks
KP = 113      # partitions per chunk: 112 real k (kh%4, kw) + 1 bias row

# location blocks per core: pair-aligned (pairs share a PSUM bank)
BLOCKS = [(0, 4), (4, 8), (8, 13)]

_CACHED = {}


def _build_bass():
    nc = bass.Bass(trn_type="TRN2")
    xk = nc.dram_tensor("xk", [KP, LPC, KC, B], mybir.dt.float16,
                        kind="ExternalInput")
    wk = nc.dram_tensor("wk", [KP, LPC, KC, NF], mybir.dt.float16,
                        kind="ExternalInput")
    # separate store tensors: avoids per-tensor WAW chaining between stores
    outs = [nc.dram_tensor(f"out{i}", [NF, l1 - l0, B], mybir.dt.float32,
                           kind="ExternalOutput")
            for i, (l0, l1) in enumerate([(0, 8), (8, 13)])]

    NPAIR = (LPC + 1) // 2

    with TileContext(nc) as tc:
        with (
            tc.tile_pool(name="xp", bufs=len(BLOCKS)) as xpool,
            tc.tile_pool(name="wp", bufs=len(BLOCKS)) as wpool,
            tc.tile_pool(name="op", bufs=2) as opool,
            tc.tile_pool(name="cr", bufs=2) as crpool,
            # 2 locations share one PSUM bank: NPAIR=7 tiles <= 8 banks, so
            # banks are never reused and matmuls need no release wait.
            tc.tile_pool(name="ps", bufs=NPAIR, space="PSUM") as pspool,
        ):
            x_ts, w_ts = {}, {}
            for (l0, l1) in BLOCKS:
                nl = l1 - l0
                x_t = xpool.tile([KP, nl, KC, B], mybir.dt.float16, tag="x")
                w_t = wpool.tile([KP, nl, KC, NF], mybir.dt.float16, tag="w")
                nc.gpsimd.dma_start(x_t[:], xk[:, l0:l1])
                nc.gpsimd.dma_start(w_t[:], wk[:, l0:l1])
                for l in range(l0, l1):
                    x_ts[l] = x_t[:, l - l0]
                    w_ts[l] = w_t[:, l - l0]

            o_ts = {}
            for i, (l0, l1) in enumerate([(0, 8), (8, 13)]):
                o_t = opool.tile([NF, l1 - l0, B], mybir.dt.float32, tag="o")
                for l in range(l0, l1):
                    o_ts[l] = o_t[:, l - l0]
                for p in range(l0 // 2, (l1 + 1) // 2):
                    pl0, pl1 = 2 * p, min(2 * p + 2, LPC)
                    ps = pspool.tile([NF, pl1 - pl0, B], mybir.dt.float32)
                    for j, l in enumerate(range(pl0, pl1)):
                        for c in range(KC):
                            nc.tensor.matmul(ps[:, j, :], w_ts[l][:, c, :],
                                             x_ts[l][:, c, :],
                                             start=(c == 0),
                                             stop=(c == KC - 1))
                    nc.vector.tensor_copy(o_t[:, pl0 - l0:pl1 - l0, :], ps[:])
                # Carrier: a Pool-engine read of o_t makes the Pool engine
                # wait on the DVE copies, so the store DMA below (also Pool)
                # gets that wait elided and stays within 1 sync-wait.
                scratch = crpool.tile([1, 2], mybir.dt.float32, tag="scratch")
                nc.gpsimd.tensor_copy(scratch[:], o_t[0:1, 0, 0:2])
                nc.gpsimd.dma_start(outs[i][:], o_t[:])
    return nc


def _pack_inputs(x, weight, bias):
    # x: [B,1,280,280] f32.  rows = i*28 + kh, kh = c*4 + khm; cols = j*28 + kw
    # xk[p, l=(i,j), c, b] fp16 with p = khm*28 + kw for p<112, p=112 -> 1.0
    xh = x.astype(np.float16).reshape(B, HS, KC, 4, WS, KW)
    # (b, i, c, khm, j, kw) -> (khm, kw, i, j, c, b)
    xt = np.ascontiguousarray(xh.transpose(3, 5, 1, 4, 2, 0))
    xt = xt.reshape(112, L, KC, B)
    xkf = np.empty((KP, LPAD, KC, B), np.float16)
    xkf[:112, :L] = xt
    xkf[:112, L:] = 0
    xkf[112] = 1.0

    # weight: [NF*L, 1, 28, 28] -> [f, l, c, khm, kw] -> [(khm,kw), l, c, f]
    wh = weight.astype(np.float16).reshape(NF, L, KC, 4, KW)
    wt = np.ascontiguousarray(wh.transpose(3, 4, 1, 2, 0)).reshape(112, L, KC, NF)
    wkf = np.zeros((KP, LPAD, KC, NF), np.float16)
    wkf[:112, :L] = wt
    # bias row: only chunk 0 carries it (x side is 1.0 in every chunk)
    wkf[112, :L, 0, :] = bias.astype(np.float16).reshape(NF, L).T

    in_maps = []
    for c in range(NCORES):
        sl = slice(c * LPC, (c + 1) * LPC)
        in_maps.append({"xk": np.ascontiguousarray(xkf[:, sl]),
                        "wk": np.ascontiguousarray(wkf[:, sl])})
    return in_maps


def run(x, weight, bias, **run_kwargs):
    """Build+run; returns (output, BassKernelResults)."""
    if "nc" not in _CACHED:
        _CACHED["nc"] = _build_bass()
    nc = _CACHED["nc"]
    in_maps = _pack_inputs(x, weight, bias)
    res = bass_utils.run_bass_kernel_spmd(
        nc, in_maps, core_ids=list(range(NCORES)), **run_kwargs)
    # per core: out{i} is [NF, nl, B]; concat -> [NF, LPC, B]
    outs = np.stack([
        np.concatenate([r["out0"], r["out1"]], axis=1)
        for r in res.results])                        # [8, NF, LPC, B]
    outs = outs.transpose(0, 2, 1, 3).reshape(LPAD, NF, B)[:L]  # [l, f, b]
    out = np.ascontiguousarray(outs.transpose(2, 1, 0)).reshape(B, NF, HS, WS)
    return out.astype(np.float32), res


def kernel(x, weight, bias):
    out, _ = run(x, weight, bias)
    return out
